# revision 46
# baseline (speedup 1.0000x reference)
"""GAT (2-layer, PyG-style) on 8 Trainium2 NeuronCores via Bass/Tile.

Strategy (dst-sharded message passing, bf16, descriptor-minimized):
  - Destination nodes partitioned into 8 contiguous chunks (6250/core); each
    core owns all edges incident to its dst chunk, grouped into 128-dst
    windows. Per (window, src-half) the edge list is padded only to the
    max-over-cores count M (dup-pad with idx 0, then -1 skip markers), so
    the SWDGE gather generates ~M descriptors, not ceil128(M) -- the Q7
    descriptor generation on GPSIMD is the kernel's critical path.
  - Self-loop edges (one per dst) get a dedicated chunk per window, filled
    by a direct dynamic-offset HWDGE DMA instead of gather descriptors.
  - Phase 1 (replicated): h_ext = x @ [W1 | W1@Asrc | W1@Adst] in bf16 ->
    h_tab [npad, 384] bf16 (768B rows: h(256) | aS(8) | aD(8) | pad).
  - Per-edge aD is NOT gathered: the host ships a one-hot ST matrix
    [dst(128part), edge] per chunk; tiny PE matmuls ST_k^T @ aD_win
    broadcast the 128 per-dst values to edge slots. The same windows' S
    matrix (edge-part layout, built on-vector via is_equal) drives the
    segment softmax + weighted aggregation, with the denominator riding the
    same matmul (p written over the aS columns).
  - Layer-2 rows [h2(64) | aS2 | aD2] come from one bf16 matmul per window;
    [h2|aS2] -> h2_mine (256B rows), aD2 -> SBUF. One AllGather (bf16).
  - Phase 3: same machinery, single head; aS2 rides in the gathered row,
    aD2 via ST matmul; log_softmax with Exp(bias=-max, accum) on scalar.
"""
import sys

for _p in ("/opt/trn_rl_repo", "/opt/pypackages"):
    if _p not in sys.path:
        sys.path.insert(0, _p)

import numpy as np
from concourse import bacc, bass, mybir, tile
from concourse.masks import make_identity

P = 128
F32 = mybir.dt.float32
BF16 = mybir.dt.bfloat16
I16 = mybir.dt.int16
HALF = 32768
SENT = 200.0  # d128 sentinel for pad slots (never matches iota 0..127)

# ---- problem constants (nn_GAT_60000693125135) ----
N = 50000
IN_DIM = 256
H1 = 8          # heads layer 1
HID = 32        # per-head dim layer 1
HC1 = H1 * HID  # 256
OUT = 64
NCORES = 8
NEG_SLOPE = 0.2
TROW = 384      # h_tab bf16 row stride (768B); cols 0:272 used
ECOL = HC1 + 2 * H1  # 272


def _cdiv(a, b):
    return -(-a // b)


def _wrap16(vals, nidx):
    """int16 idx list -> [128, nidx//16] wrap-16 layout, replicated x8."""
    a = np.asarray(vals, np.int16).reshape(nidx // 16, 16).T  # [16, cols]
    return np.tile(a, (8, 1))


# ----------------------------------------------------------------------------
# Host-side preprocessing.
# ----------------------------------------------------------------------------
def prep_edges(edge_index, n, ncores, B=2):
    """Shard + window + src-half-split the (non-self-loop) edge list, then
    merge gather calls across window PAIRS.

    Pair chunk layout: [lo(a1) lo(a2) | hi(b1) hi(b2) | self(w0) self(w1)]
    where (a1, a2) orders the pair's windows so the one with the larger lo
    pad goes last (its -1 tail is trimmed via num_idxs_reg); likewise
    (b1, b2) for hi. Self-loops get dedicated chunks (direct DMA, d128 =
    iota). Dup-pads use idx 0 with d128 = SENT.

    Returns per-core srclo16/srchi16/d128/ST plus a per-pair static plan.
    """
    src = edge_index[0].astype(np.int64)
    dst = edge_index[1].astype(np.int64)

    nchunk = n // ncores
    nw = _cdiv(nchunk, P)
    core = dst // nchunk
    dloc = dst - core * nchunk
    w = dloc // P
    hi = (src >= HALF).astype(np.int64)
    gid = (core * nw + w) * 2 + hi
    ngroups = ncores * nw * 2
    cnt = np.bincount(gid, minlength=ngroups).reshape(ncores, nw, 2)
    M = cnt.max(axis=0)                       # [nw, 2]
    Clo = _cdiv(M[:, 0], P)
    Chi = _cdiv(M[:, 1], P)

    order = np.argsort(gid, kind="stable")
    gid_s = gid[order]
    starts = np.concatenate([[0], np.cumsum(np.bincount(gid_s, minlength=ngroups))])
    pos = np.arange(order.size) - starts[gid_s]
    c_s, w_s, hi_s = core[order], w[order], hi[order]
    slot = pos + hi_s * (Clo[w_s] * P)

    # per-window slot arrays: [lo chunks | hi chunks]
    cmaxw = int((Clo + Chi).max())
    srcv = np.full((ncores, nw, cmaxw * P), -1, np.int64)
    d128v = np.full((ncores, nw, cmaxw * P), SENT, np.float64)
    srcv[c_s, w_s, slot] = src[order] - hi_s * HALF
    d128v[c_s, w_s, slot] = dloc[order] % P
    for wi in range(nw):
        for h in range(2):
            m = int(M[wi, h])
            base = 0 if h == 0 else int(Clo[wi]) * P
            for c in range(ncores):
                k = int(cnt[c, wi, h])
                if k < m:
                    srcv[c, wi, base + k:base + m] = 0  # dup-pad: valid idx

    # ---- build pairs ----
    pairs = []
    for p0 in range(0, nw, B):
        wlist = list(range(p0, min(p0 + B, nw)))
        lo_order = sorted(wlist, key=lambda x: -(Clo[x] * P - M[x, 0]))[::-1]
        hi_order = sorted(wlist, key=lambda x: -(Chi[x] * P - M[x, 1]))[::-1]
        pairs.append(dict(wlist=wlist, lo_order=lo_order, hi_order=hi_order))

    slo_cols, shi_cols, d128_cols, st_cols = [], [], [], []
    olo8 = ohi8 = oall = 0
    for pr in pairs:
        wlist, lo_o, hi_o = pr["wlist"], pr["lo_order"], pr["hi_order"]
        CloT = int(sum(Clo[x] for x in wlist))
        ChiT = int(sum(Chi[x] for x in wlist))
        Cp = CloT + ChiT + len(wlist)
        # chunk offsets per window within the pair tile
        lo_off, off = {}, 0
        for x in lo_o:
            lo_off[x] = off; off += int(Clo[x])
        hi_off = {}
        for x in hi_o:
            hi_off[x] = off; off += int(Chi[x])
        self_off = {}
        for x in wlist:
            self_off[x] = off; off += 1
        assert off == Cp
        # merged index lists (slots follow chunk order lo then hi)
        lo_list = [srcv[:, x, 0:int(Clo[x]) * P] for x in lo_o]
        hi_list = [srcv[:, x, int(Clo[x]) * P:int(Clo[x] + Chi[x]) * P]
                   for x in hi_o]
        lo_cat = (np.concatenate(lo_list, axis=1)
                  if lo_list else np.zeros((ncores, 0), np.int64))
        hi_cat = (np.concatenate(hi_list, axis=1)
                  if hi_list else np.zeros((ncores, 0), np.int64))
        # trim: all but the LAST window's tail must be valid (dup-pad them)
        for cat, olist, Cx, Mx in ((lo_cat, lo_o, Clo, M[:, 0]),
                                   (hi_cat, hi_o, Chi, M[:, 1])):
            off2 = 0
            for i, x in enumerate(olist):
                nfull = int(Cx[x]) * P
                if i < len(olist) - 1:
                    seg = cat[:, off2:off2 + nfull]
                    seg[seg < 0] = 0          # mid-call pads must be valid
                off2 += nfull
        reg_lo = (CloT - (int(Clo[lo_o[-1]]) if lo_o else 0)) * P + (
            int(M[lo_o[-1], 0]) if lo_o else 0)
        reg_hi = (ChiT - (int(Chi[hi_o[-1]]) if hi_o else 0)) * P + (
            int(M[hi_o[-1], 1]) if hi_o else 0)
        # d128 per chunk order
        dcols = np.full((ncores, Cp * P), SENT, np.float64)
        for x in wlist:
            nlo = int(Clo[x]) * P
            dcols[:, lo_off[x] * P:lo_off[x] * P + nlo] = d128v[:, x, 0:nlo]
            nhi = int(Chi[x]) * P
            dcols[:, hi_off[x] * P:hi_off[x] * P + nhi] = (
                d128v[:, x, nlo:nlo + nhi])
            rows = min(P, nchunk - x * P)
            dcols[:, self_off[x] * P:self_off[x] * P + rows] = np.arange(rows)
        slo_cols.append(lo_cat)
        shi_cols.append(hi_cat)
        d128_cols.append(dcols)
        pr.update(CloT=CloT, ChiT=ChiT, Cp=Cp, lo_off=lo_off, hi_off=hi_off,
                  self_off=self_off, reg_lo=int(reg_lo), reg_hi=int(reg_hi),
                  olo=olo8, ohi=ohi8, oall=oall,
                  Clo={x: int(Clo[x]) for x in wlist},
                  Chi={x: int(Chi[x]) for x in wlist})
        olo8 += CloT; ohi8 += ChiT; oall += Cp

    CTlo, CThi, CT = olo8, ohi8, oall
    lo_all = np.concatenate(slo_cols, axis=1)
    hi_all = np.concatenate(shi_cols, axis=1)
    d_all = np.concatenate(d128_cols, axis=1)     # [ncores, CT*P] slot-major
    srclo16 = np.full((ncores, P, max(8 * CTlo, 16)), -1, np.int16)
    srchi16 = np.full((ncores, P, max(8 * CThi, 16)), -1, np.int16)
    for c in range(ncores):
        if CTlo:
            srclo16[c, :, 0:8 * CTlo] = _wrap16(lo_all[c], CTlo * P)
        if CThi:
            srchi16[c, :, 0:8 * CThi] = _wrap16(hi_all[c], CThi * P)
    # d128 [P, CT] chunk-major-transposed; ST one-hot [P, CT*P]
    import ml_dtypes
    STh = np.zeros((ncores, P, CT * P), ml_dtypes.bfloat16)
    Sh = np.zeros((ncores, P, CT * P), ml_dtypes.bfloat16)
    iot = np.arange(P)
    for c in range(ncores):
        STh[c] = (d_all[c][None, :] == iot[:, None])
        blocks = (d_all[c].reshape(CT, P)[:, :, None] == iot[None, None, :])
        Sh[c] = blocks.transpose(1, 0, 2).reshape(P, CT * P)
    return dict(srclo16=srclo16, srchi16=srchi16, ST=STh, S=Sh,
                pairs=pairs, CTlo=CTlo, CThi=CThi, CT=CT,
                cmaxp=max(pr["Cp"] for pr in pairs))


# ----------------------------------------------------------------------------
# Kernel builder (SPMD program, same for all cores).
# ----------------------------------------------------------------------------
def build_nc(cfg):
    n = cfg["N"]; in_dim = cfg["IN"]; hc1 = cfg["HC1"]; h1 = cfg["H1"]
    hid = cfg["HID"]; out_dim = cfg["OUT"]; ncores = cfg["NCORES"]
    neg = cfg["NEG"]
    pairs = cfg["pairs"]
    CTlo, CThi, CT = cfg["CTlo"], cfg["CThi"], cfg["CT"]
    cmax = cfg["cmaxp"]

    nchunk = n // ncores
    nw = _cdiv(nchunk, P)
    ntiles = _cdiv(n, P)
    npad = ntiles * P
    kt1 = _cdiv(in_dim, P)
    NB = 8
    L2C = 2 * out_dim            # h2_mine row stride (bf16) -> 256B

    nc = bacc.Bacc(None, target_bir_lowering=False, debug=False,
                   num_devices=ncores)

    # ---- I/O ----
    xT_in = nc.dram_tensor("xT", [in_dim, npad], BF16, kind="ExternalInput")
    w1e_in = nc.dram_tensor("W1ext", [in_dim, ECOL], BF16, kind="ExternalInput")
    w2e_in = nc.dram_tensor("W2ext", [hc1, out_dim + 2], BF16,
                            kind="ExternalInput")
    b1r_in = nc.dram_tensor("b1r", [P, hc1], F32, kind="ExternalInput")
    b2r_in = nc.dram_tensor("b2r", [P, out_dim], F32, kind="ExternalInput")
    slo_in = nc.dram_tensor("srclo16", [P, max(8 * CTlo, 16)], I16,
                            kind="ExternalInput")
    shi_in = nc.dram_tensor("srchi16", [P, max(8 * CThi, 16)], I16,
                            kind="ExternalInput")
    st_in = nc.dram_tensor("STh", [P, CT * P], BF16, kind="ExternalInput")
    s_in = nc.dram_tensor("Sh", [P, CT * P], BF16, kind="ExternalInput")
    out_ext = nc.dram_tensor("out", [nchunk, out_dim], F32,
                             kind="ExternalOutput")

    with tile.TileContext(nc) as tc:
        with (
            tc.tile_pool(name="dram", bufs=1, space="DRAM") as dram,
            tc.tile_pool(name="const", bufs=1) as cpool,
            tc.tile_pool(name="gbuf", bufs=3) as gpool,
            tc.tile_pool(name="g2buf", bufs=2) as g2pool,
            tc.tile_pool(name="p2buf", bufs=1) as p2pool,
            tc.tile_pool(name="stbuf", bufs=2) as stpool,
            tc.tile_pool(name="sbuf2", bufs=2) as spool,
            tc.tile_pool(name="small", bufs=3) as smpool,
            tc.tile_pool(name="psA", bufs=2, space="PSUM") as psA,
            tc.tile_pool(name="psB", bufs=2, space="PSUM") as psB,
            tc.tile_pool(name="psC", bufs=2, space="PSUM") as psC,
        ):
            # ---- DRAM scratch ----
            h_tab = dram.tile([npad, TROW], BF16)
            h2_mine = dram.tile([nchunk, L2C], BF16)
            h2_tab = dram.tile([n, L2C], BF16, addr_space="Shared")

            # ---- resident constants ----
            identB = cpool.tile([P, P], BF16)
            make_identity(nc, identB[:])
            b1r = cpool.tile([P, hc1], F32)
            nc.sync.dma_start(out=b1r[:], in_=b1r_in[:])
            b2r = cpool.tile([P, out_dim], F32)
            nc.sync.dma_start(out=b2r[:], in_=b2r_in[:])
            slo = cpool.tile([P, max(8 * CTlo, 16)], I16)
            nc.sync.dma_start(out=slo[:], in_=slo_in[:])
            shi = cpool.tile([P, max(8 * CThi, 16)], I16)
            nc.sync.dma_start(out=shi[:], in_=shi_in[:])
            w1e = cpool.tile([P, kt1, ECOL], BF16)
            for kt in range(kt1):
                kp = min(P, in_dim - kt * P)
                nc.sync.dma_start(out=w1e[:kp, kt, :],
                                  in_=w1e_in[kt * P:kt * P + kp, :])
            ckt = _cdiv(hc1, P)
            w2e = cpool.tile([P, ckt, out_dim + 2], BF16)
            for c in range(ckt):
                cp = min(P, hc1 - c * P)
                nc.sync.dma_start(out=w2e[:cp, c, :],
                                  in_=w2e_in[c * P:c * P + cp, :])
            zeros64 = cpool.tile([P, out_dim], F32)
            nc.vector.memset(zeros64[:], 0.0)
            zeros256 = cpool.tile([P, hc1], F32)
            nc.vector.memset(zeros256[:], 0.0)
            zband = cpool.tile([P, cmax, h1], BF16)
            nc.vector.memset(zband[:], 0.0)

            # ---- phase 1: h_ext = x @ w1ext -> h_tab (bf16) ----
            with (
                tc.tile_pool(name="xst", bufs=2) as xpool,
                tc.tile_pool(name="hst", bufs=2) as hpool,
            ):
                for g in range(_cdiv(ntiles, NB)):
                    nt0 = g * NB
                    nb = min(NB, ntiles - nt0)
                    xst = xpool.tile([P, kt1, NB * P], BF16, tag="xst")
                    for kt in range(kt1):
                        kp = min(P, in_dim - kt * P)
                        nc.sync.dma_start(
                            out=xst[:kp, kt, 0:nb * P],
                            in_=xT_in[kt * P:kt * P + kp,
                                      nt0 * P:nt0 * P + nb * P])
                    hstg = hpool.tile([P, NB, ECOL], BF16, tag="hst")
                    for j in range(nb):
                        ps = psA.tile([P, ECOL], F32, tag="mm")
                        for kt in range(kt1):
                            kp = min(P, in_dim - kt * P)
                            nc.tensor.matmul(
                                out=ps[:], lhsT=xst[:kp, kt, j * P:(j + 1) * P],
                                rhs=w1e[:kp, kt, :],
                                start=(kt == 0), stop=(kt == kt1 - 1))
                        if j % 2 == 0:
                            nc.scalar.copy(out=hstg[:, j, :], in_=ps[:])
                        else:
                            nc.vector.tensor_copy(hstg[:, j, :], ps[:])
                    hv = h_tab[nt0 * P:(nt0 + nb) * P, 0:ECOL].rearrange(
                        "(j p) c -> p j c", p=P)
                    nc.sync.dma_start(out=hv, in_=hstg[:, 0:nb, :])

            # ---- own-node aD rows -> SBUF (dynamic-offset HWDGE DMA) ----
            pid_rows = nc.sync.snap(nc.sync.partition_id() * nchunk)
            adl = cpool.tile([P, nw, h1], BF16)
            a2l = cpool.tile([P, nw], BF16)
            nc.vector.memset(adl[:], 0.0)
            nc.vector.memset(a2l[:], 0.0)
            nwf = nchunk // P          # full windows
            nc.sync.dma_start(
                out=adl[:, 0:nwf, :],
                in_=h_tab[bass.ds(pid_rows, nwf * P), hc1 + h1:ECOL].rearrange(
                    "(w p) c -> p w c", p=P))
            lrows = nchunk - nwf * P
            if lrows:
                nc.sync.dma_start(
                    out=adl[:lrows, nwf, :],
                    in_=h_tab[bass.ds(pid_rows + nwf * P, lrows),
                              hc1 + h1:ECOL])

            stop = cfg.get("STOP", "")

            def bounce_out(src_dram, cols):
                for w in range(nw):
                    rows = min(P, nchunk - w * P)
                    dbgb = smpool.tile([P, out_dim], BF16, tag="zb")
                    nc.vector.memset(dbgb[:], 0.0)
                    nc.sync.dma_start(out=dbgb[:rows, 0:cols],
                                      in_=src_dram[w * P:w * P + rows, 0:cols])
                    dbg = smpool.tile([P, out_dim], F32, tag="z")
                    nc.vector.tensor_copy(dbg[:], dbgb[:])
                    nc.sync.dma_start(out=out_ext[w * P:w * P + rows, :],
                                      in_=dbg[:rows, :])

            if stop == "phase1":
                bounce_out(h_tab, out_dim)
                return nc

            # ---- phase 2: layer-1 edge aggregation per dst window pair ----
            def chunks_of(pr, x):
                return (list(range(pr["lo_off"][x], pr["lo_off"][x] + pr["Clo"][x]))
                        + list(range(pr["hi_off"][x], pr["hi_off"][x] + pr["Chi"][x]))
                        + [pr["self_off"][x]])

            for i in range(3):
                gi = gpool.tile([P, cmax, TROW], BF16, tag="G")
                nc.vector.memset(gi[:], 0.0)
            G_cur = gpool.tile([P, cmax, TROW], BF16, tag="G")
            nc.scalar.copy(out=G_cur[:, 0:pairs[0]["Cp"], hc1:hc1 + h1],
                           in_=zband[:, 0:pairs[0]["Cp"], :])
            for ip, pr in enumerate(pairs):
                wlist, Cp = pr["wlist"], pr["Cp"]
                CloT, ChiT = pr["CloT"], pr["ChiT"]
                oall = pr["oall"]
                G = G_cur
                if pr["reg_lo"]:
                    nc.gpsimd.dma_gather(
                        out_ap=G[:, 0:CloT, :], in_ap=h_tab[:],
                        idxs_ap=slo[:, 8 * pr["olo"]:8 * (pr["olo"] + CloT)],
                        num_idxs=CloT * P, num_idxs_reg=pr["reg_lo"],
                        elem_size=TROW, single_packet=False)
                if pr["reg_hi"]:
                    nc.gpsimd.dma_gather(
                        out_ap=G[:, CloT:CloT + ChiT, :], in_ap=h_tab[HALF:, :],
                        idxs_ap=shi[:, 8 * pr["ohi"]:8 * (pr["ohi"] + ChiT)],
                        num_idxs=ChiT * P, num_idxs_reg=pr["reg_hi"],
                        elem_size=TROW, single_packet=False)
                for x in wlist:
                    rows = min(P, nchunk - x * P)
                    nc.sync.dma_start(
                        out=G[:rows, pr["self_off"][x], 0:ECOL],
                        in_=h_tab[bass.ds(pid_rows + x * P, rows), 0:ECOL])
                if ip + 1 < len(pairs):
                    cpn = pairs[ip + 1]["Cp"]
                    G_cur = gpool.tile([P, cmax, TROW], BF16, tag="G")
                    nc.scalar.copy(out=G_cur[:, 0:cpn, hc1:hc1 + h1],
                                   in_=zband[:, 0:cpn, :])
                STw = stpool.tile([P, cmax, P], BF16, tag="ST")
                nc.sync.dma_start(out=STw[:, 0:Cp, :],
                                  in_=st_in[:, P * oall:P * (oall + Cp)])
                S = spool.tile([P, cmax, P], BF16, tag="S")
                nc.scalar.dma_start(out=S[:, 0:Cp, :],
                                    in_=s_in[:, P * oall:P * (oall + Cp)])
                aDps = psB.tile([P, cmax, h1], F32, tag="aD")
                for x in wlist:
                    for k in chunks_of(pr, x):
                        nc.tensor.matmul(out=aDps[:, k, :], lhsT=STw[:, k, :],
                                         rhs=adl[:, x, :], start=True, stop=True)
                aDsb = smpool.tile([P, cmax, h1], BF16, tag="aDsb")
                nc.scalar.copy(out=aDsb[:, 0:Cp, :], in_=aDps[:, 0:Cp, :])
                # e = lrelu(aS + aD); p = exp(e) written over the aS columns
                nc.vector.tensor_add(out=G[:, 0:Cp, hc1:hc1 + h1],
                                     in0=G[:, 0:Cp, hc1:hc1 + h1],
                                     in1=aDsb[:, 0:Cp, :])
                nc.vector.scalar_tensor_tensor(
                    out=G[:, 0:Cp, hc1:hc1 + h1],
                    in0=G[:, 0:Cp, hc1:hc1 + h1], scalar=neg,
                    in1=G[:, 0:Cp, hc1:hc1 + h1],
                    op0=mybir.AluOpType.mult, op1=mybir.AluOpType.max)
                nc.scalar.activation(out=G[:, 0:Cp, hc1:hc1 + h1],
                                     in_=G[:, 0:Cp, hc1:hc1 + h1],
                                     func=mybir.ActivationFunctionType.Exp)
                # value cols are j-major (host-permuted): [e, k, j, h]
                g4 = G[:, 0:Cp, 0:hc1].rearrange("p k (j h) -> p k j h", h=h1)
                nc.vector.tensor_tensor(
                    out=g4, in0=g4,
                    in1=G[:, 0:Cp, hc1:hc1 + h1].unsqueeze(2).to_broadcast(
                        (P, Cp, hid, h1)),
                    op=mybir.AluOpType.mult)
                for x in wlist:
                    rows = min(P, nchunk - x * P)
                    kl = chunks_of(pr, x)
                    ops = psA.tile([P, hc1 + h1], F32, tag="mm")
                    for i, k in enumerate(kl):
                        nc.tensor.matmul(out=ops[:], lhsT=S[:, k, :],
                                         rhs=G[:, k, 0:hc1 + h1],
                                         start=(i == 0), stop=(i == len(kl) - 1))
                    rec = smpool.tile([P, h1], F32, tag="rec")
                    nc.vector.reciprocal(out=rec[:], in_=ops[:, hc1:hc1 + h1])
                    t1 = smpool.tile([P, hc1], F32, tag="t1")
                    nc.vector.tensor_tensor(
                        out=t1[:].rearrange("p (j h) -> p j h", h=h1),
                        in0=ops[:, 0:hc1].rearrange("p (j h) -> p j h", h=h1),
                        in1=rec[:].unsqueeze(1).to_broadcast((P, hid, h1)),
                        op=mybir.AluOpType.mult)
                    nc.vector.tensor_add(out=t1[:], in0=t1[:], in1=b1r[:])
                    h1w = spool.tile([P, hc1], BF16, tag="h1w")
                    nc.vector.tensor_tensor(out=h1w[:], in0=t1[:],
                                            in1=zeros256[:],
                                            op=mybir.AluOpType.max)
                    # layer-2 row prep: [h2 | aS2 | aD2] = h1 @ w2ext
                    h1T = spool.tile([P, ckt, P], BF16, tag="h1T")
                    for c in range(ckt):
                        tp = psB.tile([P, P], BF16, tag="tp")
                        nc.tensor.transpose(tp[:], h1w[:, c * P:(c + 1) * P],
                                            identB[:])
                        nc.scalar.copy(out=h1T[:, c, :], in_=tp[:])
                    h2ps = psC.tile([P, out_dim + 2], F32, tag="h2")
                    for c in range(ckt):
                        nc.tensor.matmul(out=h2ps[:], lhsT=h1T[:, c, :],
                                         rhs=w2e[:, c, :],
                                         start=(c == 0), stop=(c == ckt - 1))
                    h2sb = smpool.tile([P, out_dim + 2], BF16, tag="h2sb")
                    nc.scalar.copy(out=h2sb[:], in_=h2ps[:])
                    nc.sync.dma_start(
                        out=h2_mine[x * P:x * P + rows, 0:out_dim + 1],
                        in_=h2sb[:rows, 0:out_dim + 1])
                    nc.scalar.copy(out=a2l[:rows, x:x + 1],
                                   in_=h2ps[:rows, out_dim + 1:out_dim + 2])

            if stop == "phase2":
                bounce_out(h2_mine, out_dim)
                return nc

            # ---- all-gather h2 ----
            nc.gpsimd.collective_compute(
                "AllGather", mybir.AluOpType.bypass,
                replica_groups=[list(range(ncores))],
                ins=[h2_mine[:].opt()], outs=[h2_tab[:].opt()])

            # ---- phase 3: layer-2 edge aggregation + log_softmax ----
            for i in range(2):
                gi = g2pool.tile([P, cmax, L2C], BF16, tag="G2")
                nc.vector.memset(gi[:], 0.0)
            t_all = cpool.tile([P, nw, out_dim], BF16)
            s_all = cpool.tile([P, nw], F32)
            G2_cur = g2pool.tile([P, cmax, L2C], BF16, tag="G2")
            nc.scalar.copy(out=G2_cur[:, 0:pairs[0]["Cp"], out_dim:out_dim + 1],
                           in_=zband[:, 0:pairs[0]["Cp"], 0:1])
            for ip, pr in enumerate(pairs):
                wlist, Cp = pr["wlist"], pr["Cp"]
                CloT, ChiT = pr["CloT"], pr["ChiT"]
                oall = pr["oall"]
                G2 = G2_cur
                if pr["reg_lo"]:
                    nc.gpsimd.dma_gather(
                        out_ap=G2[:, 0:CloT, :], in_ap=h2_tab[:],
                        idxs_ap=slo[:, 8 * pr["olo"]:8 * (pr["olo"] + CloT)],
                        num_idxs=CloT * P, num_idxs_reg=pr["reg_lo"],
                        elem_size=L2C, single_packet=False)
                if pr["reg_hi"]:
                    nc.gpsimd.dma_gather(
                        out_ap=G2[:, CloT:CloT + ChiT, :], in_ap=h2_tab[HALF:, :],
                        idxs_ap=shi[:, 8 * pr["ohi"]:8 * (pr["ohi"] + ChiT)],
                        num_idxs=ChiT * P, num_idxs_reg=pr["reg_hi"],
                        elem_size=L2C, single_packet=False)
                for x in wlist:
                    rows = min(P, nchunk - x * P)
                    nc.sync.dma_start(
                        out=G2[:rows, pr["self_off"][x], 0:out_dim + 1],
                        in_=h2_mine[x * P:x * P + rows, 0:out_dim + 1])
                if ip + 1 < len(pairs):
                    cpn = pairs[ip + 1]["Cp"]
                    G2_cur = g2pool.tile([P, cmax, L2C], BF16, tag="G2")
                    nc.scalar.copy(out=G2_cur[:, 0:cpn, out_dim:out_dim + 1],
                                   in_=zband[:, 0:cpn, 0:1])
                STw = stpool.tile([P, cmax, P], BF16, tag="ST")
                nc.sync.dma_start(out=STw[:, 0:Cp, :],
                                  in_=st_in[:, P * oall:P * (oall + Cp)])
                S = spool.tile([P, cmax, P], BF16, tag="S")
                nc.scalar.dma_start(out=S[:, 0:Cp, :],
                                    in_=s_in[:, P * oall:P * (oall + Cp)])
                aD2ps = psB.tile([P, cmax], F32, tag="aD")
                for x in wlist:
                    for k in chunks_of(pr, x):
                        nc.tensor.matmul(out=aD2ps[:, k:k + 1],
                                         lhsT=STw[:, k, :],
                                         rhs=a2l[:, x:x + 1],
                                         start=True, stop=True)
                aD2sb = smpool.tile([P, cmax], BF16, tag="aDsb")
                nc.scalar.copy(out=aD2sb[:, 0:Cp], in_=aD2ps[:, 0:Cp])
                nc.vector.tensor_add(out=G2[:, 0:Cp, out_dim],
                                     in0=G2[:, 0:Cp, out_dim],
                                     in1=aD2sb[:, 0:Cp])
                nc.vector.scalar_tensor_tensor(
                    out=G2[:, 0:Cp, out_dim], in0=G2[:, 0:Cp, out_dim],
                    scalar=neg, in1=G2[:, 0:Cp, out_dim],
                    op0=mybir.AluOpType.mult, op1=mybir.AluOpType.max)
                nc.scalar.activation(out=G2[:, 0:Cp, out_dim],
                                     in_=G2[:, 0:Cp, out_dim],
                                     func=mybir.ActivationFunctionType.Exp)
                p2x = p2pool.tile([P, cmax, out_dim], BF16, tag="p2x")
                nc.scalar.copy(
                    out=p2x[:, 0:Cp, :],
                    in_=G2[:, 0:Cp, out_dim:out_dim + 1].to_broadcast(
                        (P, Cp, out_dim)))
                nc.vector.tensor_tensor(
                    out=G2[:, 0:Cp, 0:out_dim], in0=G2[:, 0:Cp, 0:out_dim],
                    in1=p2x[:, 0:Cp, :],
                    op=mybir.AluOpType.mult)
                for x in wlist:
                    kl = chunks_of(pr, x)
                    ops2 = psA.tile([P, out_dim + 1], F32, tag="mm")
                    for i, k in enumerate(kl):
                        nc.tensor.matmul(out=ops2[:], lhsT=S[:, k, :],
                                         rhs=G2[:, k, 0:out_dim + 1],
                                         start=(i == 0),
                                         stop=(i == len(kl) - 1))
                    rec2 = smpool.tile([P, 1], F32, tag="rec")
                    nc.vector.reciprocal(out=rec2[:], in_=ops2[:, out_dim:])
                    z = smpool.tile([P, out_dim], F32, tag="z")
                    nc.vector.tensor_tensor(
                        out=z[:], in0=ops2[:, 0:out_dim],
                        in1=rec2[:].to_broadcast((P, out_dim)),
                        op=mybir.AluOpType.mult)
                    nc.vector.tensor_add(out=z[:], in0=z[:], in1=b2r[:])
                    negmax = smpool.tile([P, 1], F32, tag="nm")
                    nc.vector.tensor_reduce(out=negmax[:], in_=z[:],
                                            axis=mybir.AxisListType.X,
                                            op=mybir.AluOpType.max, negate=True)
                    # t = z - max (saved); s = sum(exp(t)) via scalar Exp accum
                    nc.vector.scalar_tensor_tensor(
                        out=t_all[:, x, :], in0=z[:], scalar=negmax[:],
                        in1=zeros64[:],
                        op0=mybir.AluOpType.add, op1=mybir.AluOpType.add)
                    esc = smpool.tile([P, out_dim], F32, tag="esc")
                    nc.scalar.activation(out=esc[:], in_=z[:],
                                         func=mybir.ActivationFunctionType.Exp,
                                         bias=negmax[:],
                                         accum_out=s_all[:, x:x + 1])
            # epilogue: res = t - ln(s)
            lns = cpool.tile([P, nw], F32)
            nc.scalar.activation(out=lns[:], in_=s_all[:],
                                 func=mybir.ActivationFunctionType.Ln)
            for w in range(nw):
                rows = min(P, nchunk - w * P)
                res = smpool.tile([P, out_dim], F32, tag="res")
                nc.vector.scalar_tensor_tensor(
                    out=res[:], in0=t_all[:, w, :], scalar=lns[:, w:w + 1],
                    in1=zeros64[:],
                    op0=mybir.AluOpType.subtract, op1=mybir.AluOpType.add)
                nc.sync.dma_start(out=out_ext[w * P:w * P + rows, :],
                                  in_=res[:rows, :])

    return nc


# ----------------------------------------------------------------------------
# Host-side input packing.
# ----------------------------------------------------------------------------
def make_in_maps(inputs, cfg):
    import ml_dtypes
    bf16 = ml_dtypes.bfloat16
    n = cfg["N"]; in_dim = cfg["IN"]; hc1 = cfg["HC1"]; h1 = cfg["H1"]
    hid = cfg["HID"]; out_dim = cfg["OUT"]; ncores = cfg["NCORES"]

    x = np.asarray(inputs["x"], np.float32)
    ei = np.asarray(inputs["edge_index"])
    W1 = np.asarray(inputs["W1"], np.float32)
    a_src1 = np.asarray(inputs["a_src1"], np.float32)
    a_dst1 = np.asarray(inputs["a_dst1"], np.float32)
    b1 = np.asarray(inputs["b1"], np.float32)
    W2 = np.asarray(inputs["W2"], np.float32)
    a_src2 = np.asarray(inputs["a_src2"], np.float32)
    a_dst2 = np.asarray(inputs["a_dst2"], np.float32)
    b2 = np.asarray(inputs["b2"], np.float32)

    ntiles = _cdiv(n, P)
    npad = ntiles * P
    xT = np.zeros((in_dim, npad), bf16)
    xT[:, :n] = x.T

    amat = np.zeros((hc1, 2 * h1), np.float32)
    for h in range(h1):
        amat[h * hid:(h + 1) * hid, h] = a_src1[h]
        amat[h * hid:(h + 1) * hid, h1 + h] = a_dst1[h]
    # permute hidden cols from h-major (h*hid+j) to j-major (j*h1+h) so the
    # per-head p broadcast multiplies an inner-contiguous h1-vector on DVE
    jmaj = np.arange(hc1).reshape(hid, h1)
    perm = (jmaj % h1) * hid + jmaj // h1          # new col -> old col
    perm = perm.reshape(-1)
    W1ext = np.concatenate([W1[:, perm], W1 @ amat], axis=1).astype(bf16)
    W2e_full = np.concatenate(
        [W2, (W2 @ a_src2[0])[:, None], (W2 @ a_dst2[0])[:, None]], axis=1)
    W2ext = W2e_full[perm, :].astype(bf16)                         # [256, 66]
    b1 = b1[perm]

    pe = prep_edges(ei, n, ncores)
    for k in ("pairs", "CTlo", "CThi", "CT", "cmaxp"):
        cfg[k] = pe[k]

    common = {
        "W1ext": W1ext, "W2ext": W2ext, "xT": xT,
        "b1r": np.tile(b1[None, :], (P, 1)).astype(np.float32),
        "b2r": np.tile(b2[None, :], (P, 1)).astype(np.float32),
    }
    in_maps = []
    for c in range(ncores):
        m = dict(common)
        m["srclo16"] = np.ascontiguousarray(pe["srclo16"][c])
        m["srchi16"] = np.ascontiguousarray(pe["srchi16"][c])
        m["STh"] = np.ascontiguousarray(pe["ST"][c])
        m["Sh"] = np.ascontiguousarray(pe["S"][c])
        in_maps.append(m)
    return in_maps


DEFAULT_CFG = dict(N=N, IN=IN_DIM, HC1=HC1, H1=H1, HID=HID, OUT=OUT,
                   NCORES=NCORES, NEG=NEG_SLOPE)

TRACE = False
LAST_RESULTS = None


def kernel(**inputs) -> np.ndarray:
    global LAST_RESULTS
    from concourse.bass_utils import run_bass_kernel_spmd

    cfg = dict(DEFAULT_CFG)
    in_maps = make_in_maps(inputs, cfg)
    nc = build_nc(cfg)
    if not nc.is_finalized():
        nc.finalize()
    res = run_bass_kernel_spmd(nc, in_maps, core_ids=list(range(cfg["NCORES"])),
                               trace=TRACE)
    LAST_RESULTS = res
    outs = [res.results[c]["out"] for c in range(cfg["NCORES"])]
    return np.concatenate(outs, axis=0).astype(np.float32)


# revision 48
# speedup vs baseline: 1.0166x; 1.0166x over previous
"""GAT (2-layer, PyG-style) on 8 Trainium2 NeuronCores via Bass/Tile.

Strategy (dst-sharded message passing, bf16, descriptor-minimized):
  - Destination nodes partitioned into 8 contiguous chunks (6250/core); each
    core owns all edges incident to its dst chunk, grouped into 128-dst
    windows. Per (window, src-half) the edge list is padded only to the
    max-over-cores count M (dup-pad with idx 0, then -1 skip markers), so
    the SWDGE gather generates ~M descriptors, not ceil128(M) -- the Q7
    descriptor generation on GPSIMD is the kernel's critical path.
  - Self-loop edges (one per dst) get a dedicated chunk per window, filled
    by a direct dynamic-offset HWDGE DMA instead of gather descriptors.
  - Phase 1 (replicated): h_ext = x @ [W1 | W1@Asrc | W1@Adst] in bf16 ->
    h_tab [npad, 384] bf16 (768B rows: h(256) | aS(8) | aD(8) | pad).
  - Per-edge aD is NOT gathered: the host ships a one-hot ST matrix
    [dst(128part), edge] per chunk; tiny PE matmuls ST_k^T @ aD_win
    broadcast the 128 per-dst values to edge slots. The same windows' S
    matrix (edge-part layout, built on-vector via is_equal) drives the
    segment softmax + weighted aggregation, with the denominator riding the
    same matmul (p written over the aS columns).
  - Layer-2 rows [h2(64) | aS2 | aD2] come from one bf16 matmul per window;
    [h2|aS2] -> h2_mine (256B rows), aD2 -> SBUF. One AllGather (bf16).
  - Phase 3: same machinery, single head; aS2 rides in the gathered row,
    aD2 via ST matmul; log_softmax with Exp(bias=-max, accum) on scalar.
"""
import sys

for _p in ("/opt/trn_rl_repo", "/opt/pypackages"):
    if _p not in sys.path:
        sys.path.insert(0, _p)

import numpy as np
from concourse import bacc, bass, mybir, tile
from concourse.masks import make_identity

P = 128
F32 = mybir.dt.float32
BF16 = mybir.dt.bfloat16
I16 = mybir.dt.int16
HALF = 32768
SENT = 200.0  # d128 sentinel for pad slots (never matches iota 0..127)

# ---- problem constants (nn_GAT_60000693125135) ----
N = 50000
IN_DIM = 256
H1 = 8          # heads layer 1
HID = 32        # per-head dim layer 1
HC1 = H1 * HID  # 256
OUT = 64
NCORES = 8
NEG_SLOPE = 0.2
TROW = 384      # h_tab bf16 row stride (768B); cols 0:272 used
ECOL = HC1 + 2 * H1  # 272


def _cdiv(a, b):
    return -(-a // b)


def _wrap16(vals, nidx):
    """int16 idx list -> [128, nidx//16] wrap-16 layout, replicated x8."""
    a = np.asarray(vals, np.int16).reshape(nidx // 16, 16).T  # [16, cols]
    return np.tile(a, (8, 1))


# ----------------------------------------------------------------------------
# Host-side preprocessing.
# ----------------------------------------------------------------------------
def prep_edges(edge_index, n, ncores, B=2):
    """Shard + window + src-half-split the (non-self-loop) edge list, then
    merge gather calls across window PAIRS.

    Pair chunk layout: [lo(a1) lo(a2) | hi(b1) hi(b2) | self(w0) self(w1)]
    where (a1, a2) orders the pair's windows so the one with the larger lo
    pad goes last (its -1 tail is trimmed via num_idxs_reg); likewise
    (b1, b2) for hi. Self-loops get dedicated chunks (direct DMA, d128 =
    iota). Dup-pads use idx 0 with d128 = SENT.

    Returns per-core srclo16/srchi16/d128/ST plus a per-pair static plan.
    """
    src = edge_index[0].astype(np.int64)
    dst = edge_index[1].astype(np.int64)

    nchunk = n // ncores
    nw = _cdiv(nchunk, P)
    core = dst // nchunk
    dloc = dst - core * nchunk
    w = dloc // P
    hi = (src >= HALF).astype(np.int64)
    gid = (core * nw + w) * 2 + hi
    ngroups = ncores * nw * 2
    cnt = np.bincount(gid, minlength=ngroups).reshape(ncores, nw, 2)
    M = cnt.max(axis=0)                       # [nw, 2]
    Clo = _cdiv(M[:, 0], P)
    Chi = _cdiv(M[:, 1], P)

    order = np.argsort(gid, kind="stable")
    gid_s = gid[order]
    starts = np.concatenate([[0], np.cumsum(np.bincount(gid_s, minlength=ngroups))])
    pos = np.arange(order.size) - starts[gid_s]
    c_s, w_s, hi_s = core[order], w[order], hi[order]
    slot = pos + hi_s * (Clo[w_s] * P)

    # per-window slot arrays: [lo chunks | hi chunks]
    cmaxw = int((Clo + Chi).max())
    srcv = np.full((ncores, nw, cmaxw * P), -1, np.int64)
    d128v = np.full((ncores, nw, cmaxw * P), SENT, np.float64)
    srcv[c_s, w_s, slot] = src[order] - hi_s * HALF
    d128v[c_s, w_s, slot] = dloc[order] % P
    for wi in range(nw):
        for h in range(2):
            m = int(M[wi, h])
            base = 0 if h == 0 else int(Clo[wi]) * P
            for c in range(ncores):
                k = int(cnt[c, wi, h])
                if k < m:
                    srcv[c, wi, base + k:base + m] = 0  # dup-pad: valid idx

    # ---- build pairs ----
    pairs = []
    for p0 in range(0, nw, B):
        wlist = list(range(p0, min(p0 + B, nw)))
        lo_order = sorted(wlist, key=lambda x: -(Clo[x] * P - M[x, 0]))[::-1]
        hi_order = sorted(wlist, key=lambda x: -(Chi[x] * P - M[x, 1]))[::-1]
        pairs.append(dict(wlist=wlist, lo_order=lo_order, hi_order=hi_order))

    slo_cols, shi_cols, d128_cols, st_cols = [], [], [], []
    olo8 = ohi8 = oall = 0
    for pr in pairs:
        wlist, lo_o, hi_o = pr["wlist"], pr["lo_order"], pr["hi_order"]
        CloT = int(sum(Clo[x] for x in wlist))
        ChiT = int(sum(Chi[x] for x in wlist))
        Cp = CloT + ChiT + len(wlist)
        # chunk offsets per window within the pair tile
        lo_off, off = {}, 0
        for x in lo_o:
            lo_off[x] = off; off += int(Clo[x])
        hi_off = {}
        for x in hi_o:
            hi_off[x] = off; off += int(Chi[x])
        self_off = {}
        for x in wlist:
            self_off[x] = off; off += 1
        assert off == Cp
        # merged index lists (slots follow chunk order lo then hi)
        lo_list = [srcv[:, x, 0:int(Clo[x]) * P] for x in lo_o]
        hi_list = [srcv[:, x, int(Clo[x]) * P:int(Clo[x] + Chi[x]) * P]
                   for x in hi_o]
        lo_cat = (np.concatenate(lo_list, axis=1)
                  if lo_list else np.zeros((ncores, 0), np.int64))
        hi_cat = (np.concatenate(hi_list, axis=1)
                  if hi_list else np.zeros((ncores, 0), np.int64))
        # trim: all but the LAST window's tail must be valid (dup-pad them)
        for cat, olist, Cx, Mx in ((lo_cat, lo_o, Clo, M[:, 0]),
                                   (hi_cat, hi_o, Chi, M[:, 1])):
            off2 = 0
            for i, x in enumerate(olist):
                nfull = int(Cx[x]) * P
                if i < len(olist) - 1:
                    seg = cat[:, off2:off2 + nfull]
                    seg[seg < 0] = 0          # mid-call pads must be valid
                off2 += nfull
        reg_lo = (CloT - (int(Clo[lo_o[-1]]) if lo_o else 0)) * P + (
            int(M[lo_o[-1], 0]) if lo_o else 0)
        reg_hi = (ChiT - (int(Chi[hi_o[-1]]) if hi_o else 0)) * P + (
            int(M[hi_o[-1], 1]) if hi_o else 0)
        # d128 per chunk order
        dcols = np.full((ncores, Cp * P), SENT, np.float64)
        for x in wlist:
            nlo = int(Clo[x]) * P
            dcols[:, lo_off[x] * P:lo_off[x] * P + nlo] = d128v[:, x, 0:nlo]
            nhi = int(Chi[x]) * P
            dcols[:, hi_off[x] * P:hi_off[x] * P + nhi] = (
                d128v[:, x, nlo:nlo + nhi])
            rows = min(P, nchunk - x * P)
            dcols[:, self_off[x] * P:self_off[x] * P + rows] = np.arange(rows)
        slo_cols.append(lo_cat)
        shi_cols.append(hi_cat)
        d128_cols.append(dcols)
        pr.update(CloT=CloT, ChiT=ChiT, Cp=Cp, lo_off=lo_off, hi_off=hi_off,
                  self_off=self_off, reg_lo=int(reg_lo), reg_hi=int(reg_hi),
                  olo=olo8, ohi=ohi8, oall=oall,
                  Clo={x: int(Clo[x]) for x in wlist},
                  Chi={x: int(Chi[x]) for x in wlist})
        olo8 += CloT; ohi8 += ChiT; oall += Cp

    CTlo, CThi, CT = olo8, ohi8, oall
    lo_all = np.concatenate(slo_cols, axis=1)
    hi_all = np.concatenate(shi_cols, axis=1)
    d_all = np.concatenate(d128_cols, axis=1)     # [ncores, CT*P] slot-major
    srclo16 = np.full((ncores, P, max(8 * CTlo, 16)), -1, np.int16)
    srchi16 = np.full((ncores, P, max(8 * CThi, 16)), -1, np.int16)
    for c in range(ncores):
        if CTlo:
            srclo16[c, :, 0:8 * CTlo] = _wrap16(lo_all[c], CTlo * P)
        if CThi:
            srchi16[c, :, 0:8 * CThi] = _wrap16(hi_all[c], CThi * P)
    # d128 [P, CT] chunk-major-transposed; ST one-hot [P, CT*P]
    import ml_dtypes
    STh = np.zeros((ncores, P, CT * P), ml_dtypes.bfloat16)
    Sh = np.zeros((ncores, P, CT * P), ml_dtypes.bfloat16)
    iot = np.arange(P)
    for c in range(ncores):
        STh[c] = (d_all[c][None, :] == iot[:, None])
        blocks = (d_all[c].reshape(CT, P)[:, :, None] == iot[None, None, :])
        Sh[c] = blocks.transpose(1, 0, 2).reshape(P, CT * P)
    return dict(srclo16=srclo16, srchi16=srchi16, ST=STh, S=Sh,
                pairs=pairs, CTlo=CTlo, CThi=CThi, CT=CT,
                cmaxp=max(pr["Cp"] for pr in pairs))


# ----------------------------------------------------------------------------
# Kernel builder (SPMD program, same for all cores).
# ----------------------------------------------------------------------------
def build_nc(cfg):
    n = cfg["N"]; in_dim = cfg["IN"]; hc1 = cfg["HC1"]; h1 = cfg["H1"]
    hid = cfg["HID"]; out_dim = cfg["OUT"]; ncores = cfg["NCORES"]
    neg = cfg["NEG"]
    pairs = cfg["pairs"]
    CTlo, CThi, CT = cfg["CTlo"], cfg["CThi"], cfg["CT"]
    cmax = cfg["cmaxp"]

    nchunk = n // ncores
    nw = _cdiv(nchunk, P)
    ntiles = _cdiv(n, P)
    npad = ntiles * P
    kt1 = _cdiv(in_dim, P)
    NB = 7
    L2C = 2 * out_dim            # h2_mine row stride (bf16) -> 256B

    nc = bacc.Bacc(None, target_bir_lowering=False, debug=False,
                   num_devices=ncores)

    # ---- I/O ----
    xT_in = nc.dram_tensor("xT", [in_dim, npad], BF16, kind="ExternalInput")
    w1e_in = nc.dram_tensor("W1ext", [in_dim, ECOL], BF16, kind="ExternalInput")
    w2e_in = nc.dram_tensor("W2ext", [hc1, out_dim + 2], BF16,
                            kind="ExternalInput")
    b1r_in = nc.dram_tensor("b1r", [P, hc1], F32, kind="ExternalInput")
    b2r_in = nc.dram_tensor("b2r", [P, out_dim], F32, kind="ExternalInput")
    slo_in = nc.dram_tensor("srclo16", [P, max(8 * CTlo, 16)], I16,
                            kind="ExternalInput")
    shi_in = nc.dram_tensor("srchi16", [P, max(8 * CThi, 16)], I16,
                            kind="ExternalInput")
    st_in = nc.dram_tensor("STh", [P, CT * P], BF16, kind="ExternalInput")
    s_in = nc.dram_tensor("Sh", [P, CT * P], BF16, kind="ExternalInput")
    out_ext = nc.dram_tensor("out", [nchunk, out_dim], F32,
                             kind="ExternalOutput")

    with tile.TileContext(nc) as tc:
        with (
            tc.tile_pool(name="dram", bufs=1, space="DRAM") as dram,
            tc.tile_pool(name="const", bufs=1) as cpool,
            tc.tile_pool(name="gbuf", bufs=3) as gpool,
            tc.tile_pool(name="g2buf", bufs=3) as g2pool,
            tc.tile_pool(name="p2buf", bufs=1) as p2pool,
            tc.tile_pool(name="stbuf", bufs=2) as stpool,
            tc.tile_pool(name="sbuf2", bufs=2) as spool,
            tc.tile_pool(name="small", bufs=3) as smpool,
            tc.tile_pool(name="psA", bufs=2, space="PSUM") as psA,
            tc.tile_pool(name="psB", bufs=2, space="PSUM") as psB,
            tc.tile_pool(name="psC", bufs=2, space="PSUM") as psC,
        ):
            # ---- DRAM scratch ----
            h_tab = dram.tile([npad, TROW], BF16)
            h2_mine = dram.tile([nchunk, L2C], BF16)
            h2_tab = dram.tile([n, L2C], BF16, addr_space="Shared")

            # ---- resident constants ----
            identB = cpool.tile([P, P], BF16)
            make_identity(nc, identB[:])
            b1r = cpool.tile([P, hc1], F32)
            nc.sync.dma_start(out=b1r[:], in_=b1r_in[:])
            b2r = cpool.tile([P, out_dim], F32)
            nc.sync.dma_start(out=b2r[:], in_=b2r_in[:])
            slo = cpool.tile([P, max(8 * CTlo, 16)], I16)
            nc.sync.dma_start(out=slo[:], in_=slo_in[:])
            shi = cpool.tile([P, max(8 * CThi, 16)], I16)
            nc.sync.dma_start(out=shi[:], in_=shi_in[:])
            w1e = cpool.tile([P, kt1, ECOL], BF16)
            for kt in range(kt1):
                kp = min(P, in_dim - kt * P)
                nc.sync.dma_start(out=w1e[:kp, kt, :],
                                  in_=w1e_in[kt * P:kt * P + kp, :])
            ckt = _cdiv(hc1, P)
            w2e = cpool.tile([P, ckt, out_dim + 2], BF16)
            for c in range(ckt):
                cp = min(P, hc1 - c * P)
                nc.sync.dma_start(out=w2e[:cp, c, :],
                                  in_=w2e_in[c * P:c * P + cp, :])
            zeros64 = cpool.tile([P, out_dim], F32)
            nc.vector.memset(zeros64[:], 0.0)
            zeros256 = cpool.tile([P, hc1], F32)
            nc.vector.memset(zeros256[:], 0.0)
            zband = cpool.tile([P, cmax, h1], BF16)
            nc.vector.memset(zband[:], 0.0)

            # ---- phase 1: h_ext = x @ w1ext -> h_tab (bf16) ----
            with (
                tc.tile_pool(name="xst", bufs=2) as xpool,
                tc.tile_pool(name="hst", bufs=2) as hpool,
            ):
                for g in range(_cdiv(ntiles, NB)):
                    nt0 = g * NB
                    nb = min(NB, ntiles - nt0)
                    xst = xpool.tile([P, kt1, NB * P], BF16, tag="xst")
                    for kt in range(kt1):
                        kp = min(P, in_dim - kt * P)
                        nc.sync.dma_start(
                            out=xst[:kp, kt, 0:nb * P],
                            in_=xT_in[kt * P:kt * P + kp,
                                      nt0 * P:nt0 * P + nb * P])
                    hstg = hpool.tile([P, NB, ECOL], BF16, tag="hst")
                    for j in range(nb):
                        ps = psA.tile([P, ECOL], F32, tag="mm")
                        for kt in range(kt1):
                            kp = min(P, in_dim - kt * P)
                            nc.tensor.matmul(
                                out=ps[:], lhsT=xst[:kp, kt, j * P:(j + 1) * P],
                                rhs=w1e[:kp, kt, :],
                                start=(kt == 0), stop=(kt == kt1 - 1))
                        if j % 2 == 0:
                            nc.scalar.copy(out=hstg[:, j, :], in_=ps[:])
                        else:
                            nc.vector.tensor_copy(hstg[:, j, :], ps[:])
                    hv = h_tab[nt0 * P:(nt0 + nb) * P, 0:ECOL].rearrange(
                        "(j p) c -> p j c", p=P)
                    nc.sync.dma_start(out=hv, in_=hstg[:, 0:nb, :])

            # ---- own-node aD rows -> SBUF (dynamic-offset HWDGE DMA) ----
            pid_rows = nc.sync.snap(nc.sync.partition_id() * nchunk)
            adl = cpool.tile([P, nw, h1], BF16)
            a2l = cpool.tile([P, nw], BF16)
            nc.vector.memset(adl[:], 0.0)
            nc.vector.memset(a2l[:], 0.0)
            nwf = nchunk // P          # full windows
            nc.sync.dma_start(
                out=adl[:, 0:nwf, :],
                in_=h_tab[bass.ds(pid_rows, nwf * P), hc1 + h1:ECOL].rearrange(
                    "(w p) c -> p w c", p=P))
            lrows = nchunk - nwf * P
            if lrows:
                nc.sync.dma_start(
                    out=adl[:lrows, nwf, :],
                    in_=h_tab[bass.ds(pid_rows + nwf * P, lrows),
                              hc1 + h1:ECOL])

            stop = cfg.get("STOP", "")

            def bounce_out(src_dram, cols):
                for w in range(nw):
                    rows = min(P, nchunk - w * P)
                    dbgb = smpool.tile([P, out_dim], BF16, tag="zb")
                    nc.vector.memset(dbgb[:], 0.0)
                    nc.sync.dma_start(out=dbgb[:rows, 0:cols],
                                      in_=src_dram[w * P:w * P + rows, 0:cols])
                    dbg = smpool.tile([P, out_dim], F32, tag="z")
                    nc.vector.tensor_copy(dbg[:], dbgb[:])
                    nc.sync.dma_start(out=out_ext[w * P:w * P + rows, :],
                                      in_=dbg[:rows, :])

            if stop == "phase1":
                bounce_out(h_tab, out_dim)
                return nc

            # ---- phase 2: layer-1 edge aggregation per dst window pair ----
            def chunks_of(pr, x):
                return (list(range(pr["lo_off"][x], pr["lo_off"][x] + pr["Clo"][x]))
                        + list(range(pr["hi_off"][x], pr["hi_off"][x] + pr["Chi"][x]))
                        + [pr["self_off"][x]])

            for i in range(3):
                gi = gpool.tile([P, cmax, TROW], BF16, tag="G")
                nc.vector.memset(gi[:], 0.0)
            G_cur = gpool.tile([P, cmax, TROW], BF16, tag="G")
            nc.scalar.copy(out=G_cur[:, 0:pairs[0]["Cp"], hc1:hc1 + h1],
                           in_=zband[:, 0:pairs[0]["Cp"], :])
            for ip, pr in enumerate(pairs):
                wlist, Cp = pr["wlist"], pr["Cp"]
                CloT, ChiT = pr["CloT"], pr["ChiT"]
                oall = pr["oall"]
                G = G_cur
                if pr["reg_lo"]:
                    nc.gpsimd.dma_gather(
                        out_ap=G[:, 0:CloT, :], in_ap=h_tab[:],
                        idxs_ap=slo[:, 8 * pr["olo"]:8 * (pr["olo"] + CloT)],
                        num_idxs=CloT * P, num_idxs_reg=pr["reg_lo"],
                        elem_size=TROW, single_packet=False)
                if pr["reg_hi"]:
                    nc.gpsimd.dma_gather(
                        out_ap=G[:, CloT:CloT + ChiT, :], in_ap=h_tab[HALF:, :],
                        idxs_ap=shi[:, 8 * pr["ohi"]:8 * (pr["ohi"] + ChiT)],
                        num_idxs=ChiT * P, num_idxs_reg=pr["reg_hi"],
                        elem_size=TROW, single_packet=False)
                for x in wlist:
                    rows = min(P, nchunk - x * P)
                    nc.sync.dma_start(
                        out=G[:rows, pr["self_off"][x], 0:ECOL],
                        in_=h_tab[bass.ds(pid_rows + x * P, rows), 0:ECOL])
                if ip + 1 < len(pairs):
                    cpn = pairs[ip + 1]["Cp"]
                    G_cur = gpool.tile([P, cmax, TROW], BF16, tag="G")
                    nc.scalar.copy(out=G_cur[:, 0:cpn, hc1:hc1 + h1],
                                   in_=zband[:, 0:cpn, :])
                STw = stpool.tile([P, cmax, P], BF16, tag="ST")
                nc.sync.dma_start(out=STw[:, 0:Cp, :],
                                  in_=st_in[:, P * oall:P * (oall + Cp)])
                S = spool.tile([P, cmax, P], BF16, tag="S")
                nc.scalar.dma_start(out=S[:, 0:Cp, :],
                                    in_=s_in[:, P * oall:P * (oall + Cp)])
                aDps = psB.tile([P, cmax, h1], F32, tag="aD")
                for x in wlist:
                    for k in chunks_of(pr, x):
                        nc.tensor.matmul(out=aDps[:, k, :], lhsT=STw[:, k, :],
                                         rhs=adl[:, x, :], start=True, stop=True)
                aDsb = smpool.tile([P, cmax, h1], BF16, tag="aDsb")
                nc.scalar.copy(out=aDsb[:, 0:Cp, :], in_=aDps[:, 0:Cp, :])
                # e = lrelu(aS + aD); p = exp(e) written over the aS columns
                nc.vector.tensor_add(out=G[:, 0:Cp, hc1:hc1 + h1],
                                     in0=G[:, 0:Cp, hc1:hc1 + h1],
                                     in1=aDsb[:, 0:Cp, :])
                nc.vector.scalar_tensor_tensor(
                    out=G[:, 0:Cp, hc1:hc1 + h1],
                    in0=G[:, 0:Cp, hc1:hc1 + h1], scalar=neg,
                    in1=G[:, 0:Cp, hc1:hc1 + h1],
                    op0=mybir.AluOpType.mult, op1=mybir.AluOpType.max)
                nc.scalar.activation(out=G[:, 0:Cp, hc1:hc1 + h1],
                                     in_=G[:, 0:Cp, hc1:hc1 + h1],
                                     func=mybir.ActivationFunctionType.Exp)
                # value cols are j-major (host-permuted): [e, k, j, h]
                g4 = G[:, 0:Cp, 0:hc1].rearrange("p k (j h) -> p k j h", h=h1)
                nc.vector.tensor_tensor(
                    out=g4, in0=g4,
                    in1=G[:, 0:Cp, hc1:hc1 + h1].unsqueeze(2).to_broadcast(
                        (P, Cp, hid, h1)),
                    op=mybir.AluOpType.mult)
                for x in wlist:
                    rows = min(P, nchunk - x * P)
                    kl = chunks_of(pr, x)
                    ops = psA.tile([P, hc1 + h1], F32, tag="mm")
                    for i, k in enumerate(kl):
                        nc.tensor.matmul(out=ops[:], lhsT=S[:, k, :],
                                         rhs=G[:, k, 0:hc1 + h1],
                                         start=(i == 0), stop=(i == len(kl) - 1))
                    rec = smpool.tile([P, h1], F32, tag="rec")
                    nc.vector.reciprocal(out=rec[:], in_=ops[:, hc1:hc1 + h1])
                    t1 = smpool.tile([P, hc1], F32, tag="t1")
                    nc.vector.tensor_tensor(
                        out=t1[:].rearrange("p (j h) -> p j h", h=h1),
                        in0=ops[:, 0:hc1].rearrange("p (j h) -> p j h", h=h1),
                        in1=rec[:].unsqueeze(1).to_broadcast((P, hid, h1)),
                        op=mybir.AluOpType.mult)
                    nc.vector.tensor_add(out=t1[:], in0=t1[:], in1=b1r[:])
                    h1w = spool.tile([P, hc1], BF16, tag="h1w")
                    nc.vector.tensor_tensor(out=h1w[:], in0=t1[:],
                                            in1=zeros256[:],
                                            op=mybir.AluOpType.max)
                    # layer-2 row prep: [h2 | aS2 | aD2] = h1 @ w2ext
                    h1T = spool.tile([P, ckt, P], BF16, tag="h1T")
                    for c in range(ckt):
                        tp = psB.tile([P, P], BF16, tag="tp")
                        nc.tensor.transpose(tp[:], h1w[:, c * P:(c + 1) * P],
                                            identB[:])
                        nc.scalar.copy(out=h1T[:, c, :], in_=tp[:])
                    h2ps = psC.tile([P, out_dim + 2], F32, tag="h2")
                    for c in range(ckt):
                        nc.tensor.matmul(out=h2ps[:], lhsT=h1T[:, c, :],
                                         rhs=w2e[:, c, :],
                                         start=(c == 0), stop=(c == ckt - 1))
                    h2sb = smpool.tile([P, out_dim + 2], BF16, tag="h2sb")
                    nc.scalar.copy(out=h2sb[:], in_=h2ps[:])
                    nc.sync.dma_start(
                        out=h2_mine[x * P:x * P + rows, 0:out_dim + 1],
                        in_=h2sb[:rows, 0:out_dim + 1])
                    nc.scalar.copy(out=a2l[:rows, x:x + 1],
                                   in_=h2ps[:rows, out_dim + 1:out_dim + 2])

            if stop == "phase2":
                bounce_out(h2_mine, out_dim)
                return nc

            # ---- all-gather h2 ----
            nc.gpsimd.collective_compute(
                "AllGather", mybir.AluOpType.bypass,
                replica_groups=[list(range(ncores))],
                ins=[h2_mine[:].opt()], outs=[h2_tab[:].opt()])

            # ---- phase 3: layer-2 edge aggregation + log_softmax ----
            for i in range(3):
                gi = g2pool.tile([P, cmax, L2C], BF16, tag="G2")
                nc.vector.memset(gi[:], 0.0)
            t_all = cpool.tile([P, nw, out_dim], BF16)
            s_all = cpool.tile([P, nw], F32)
            G2_cur = g2pool.tile([P, cmax, L2C], BF16, tag="G2")
            nc.scalar.copy(out=G2_cur[:, 0:pairs[0]["Cp"], out_dim:out_dim + 1],
                           in_=zband[:, 0:pairs[0]["Cp"], 0:1])
            for ip, pr in enumerate(pairs):
                wlist, Cp = pr["wlist"], pr["Cp"]
                CloT, ChiT = pr["CloT"], pr["ChiT"]
                oall = pr["oall"]
                G2 = G2_cur
                if pr["reg_lo"]:
                    nc.gpsimd.dma_gather(
                        out_ap=G2[:, 0:CloT, :], in_ap=h2_tab[:],
                        idxs_ap=slo[:, 8 * pr["olo"]:8 * (pr["olo"] + CloT)],
                        num_idxs=CloT * P, num_idxs_reg=pr["reg_lo"],
                        elem_size=L2C, single_packet=False)
                if pr["reg_hi"]:
                    nc.gpsimd.dma_gather(
                        out_ap=G2[:, CloT:CloT + ChiT, :], in_ap=h2_tab[HALF:, :],
                        idxs_ap=shi[:, 8 * pr["ohi"]:8 * (pr["ohi"] + ChiT)],
                        num_idxs=ChiT * P, num_idxs_reg=pr["reg_hi"],
                        elem_size=L2C, single_packet=False)
                for x in wlist:
                    rows = min(P, nchunk - x * P)
                    nc.sync.dma_start(
                        out=G2[:rows, pr["self_off"][x], 0:out_dim + 1],
                        in_=h2_mine[x * P:x * P + rows, 0:out_dim + 1])
                if ip + 1 < len(pairs):
                    cpn = pairs[ip + 1]["Cp"]
                    G2_cur = g2pool.tile([P, cmax, L2C], BF16, tag="G2")
                    nc.scalar.copy(out=G2_cur[:, 0:cpn, out_dim:out_dim + 1],
                                   in_=zband[:, 0:cpn, 0:1])
                STw = stpool.tile([P, cmax, P], BF16, tag="ST")
                nc.sync.dma_start(out=STw[:, 0:Cp, :],
                                  in_=st_in[:, P * oall:P * (oall + Cp)])
                S = spool.tile([P, cmax, P], BF16, tag="S")
                nc.scalar.dma_start(out=S[:, 0:Cp, :],
                                    in_=s_in[:, P * oall:P * (oall + Cp)])
                aD2ps = psB.tile([P, cmax], F32, tag="aD")
                for x in wlist:
                    for k in chunks_of(pr, x):
                        nc.tensor.matmul(out=aD2ps[:, k:k + 1],
                                         lhsT=STw[:, k, :],
                                         rhs=a2l[:, x:x + 1],
                                         start=True, stop=True)
                aD2sb = smpool.tile([P, cmax], BF16, tag="aDsb")
                nc.scalar.copy(out=aD2sb[:, 0:Cp], in_=aD2ps[:, 0:Cp])
                nc.vector.tensor_add(out=G2[:, 0:Cp, out_dim],
                                     in0=G2[:, 0:Cp, out_dim],
                                     in1=aD2sb[:, 0:Cp])
                nc.vector.scalar_tensor_tensor(
                    out=G2[:, 0:Cp, out_dim], in0=G2[:, 0:Cp, out_dim],
                    scalar=neg, in1=G2[:, 0:Cp, out_dim],
                    op0=mybir.AluOpType.mult, op1=mybir.AluOpType.max)
                nc.scalar.activation(out=G2[:, 0:Cp, out_dim],
                                     in_=G2[:, 0:Cp, out_dim],
                                     func=mybir.ActivationFunctionType.Exp)
                p2x = p2pool.tile([P, cmax, out_dim], BF16, tag="p2x")
                nc.scalar.copy(
                    out=p2x[:, 0:Cp, :],
                    in_=G2[:, 0:Cp, out_dim:out_dim + 1].to_broadcast(
                        (P, Cp, out_dim)))
                nc.vector.tensor_tensor(
                    out=G2[:, 0:Cp, 0:out_dim], in0=G2[:, 0:Cp, 0:out_dim],
                    in1=p2x[:, 0:Cp, :],
                    op=mybir.AluOpType.mult)
                for x in wlist:
                    kl = chunks_of(pr, x)
                    ops2 = psA.tile([P, out_dim + 1], F32, tag="mm")
                    for i, k in enumerate(kl):
                        nc.tensor.matmul(out=ops2[:], lhsT=S[:, k, :],
                                         rhs=G2[:, k, 0:out_dim + 1],
                                         start=(i == 0),
                                         stop=(i == len(kl) - 1))
                    rec2 = smpool.tile([P, 1], F32, tag="rec")
                    nc.vector.reciprocal(out=rec2[:], in_=ops2[:, out_dim:])
                    z = smpool.tile([P, out_dim], F32, tag="z")
                    nc.vector.tensor_tensor(
                        out=z[:], in0=ops2[:, 0:out_dim],
                        in1=rec2[:].to_broadcast((P, out_dim)),
                        op=mybir.AluOpType.mult)
                    nc.vector.tensor_add(out=z[:], in0=z[:], in1=b2r[:])
                    negmax = smpool.tile([P, 1], F32, tag="nm")
                    nc.vector.tensor_reduce(out=negmax[:], in_=z[:],
                                            axis=mybir.AxisListType.X,
                                            op=mybir.AluOpType.max, negate=True)
                    # t = z - max (saved); s = sum(exp(t)) via scalar Exp accum
                    nc.vector.scalar_tensor_tensor(
                        out=t_all[:, x, :], in0=z[:], scalar=negmax[:],
                        in1=zeros64[:],
                        op0=mybir.AluOpType.add, op1=mybir.AluOpType.add)
                    esc = smpool.tile([P, out_dim], F32, tag="esc")
                    nc.scalar.activation(out=esc[:], in_=z[:],
                                         func=mybir.ActivationFunctionType.Exp,
                                         bias=negmax[:],
                                         accum_out=s_all[:, x:x + 1])
            # epilogue: res = t - ln(s)
            lns = cpool.tile([P, nw], F32)
            nc.scalar.activation(out=lns[:], in_=s_all[:],
                                 func=mybir.ActivationFunctionType.Ln)
            for w in range(nw):
                rows = min(P, nchunk - w * P)
                res = smpool.tile([P, out_dim], F32, tag="res")
                nc.vector.scalar_tensor_tensor(
                    out=res[:], in0=t_all[:, w, :], scalar=lns[:, w:w + 1],
                    in1=zeros64[:],
                    op0=mybir.AluOpType.subtract, op1=mybir.AluOpType.add)
                nc.sync.dma_start(out=out_ext[w * P:w * P + rows, :],
                                  in_=res[:rows, :])

    return nc


# ----------------------------------------------------------------------------
# Host-side input packing.
# ----------------------------------------------------------------------------
def make_in_maps(inputs, cfg):
    import ml_dtypes
    bf16 = ml_dtypes.bfloat16
    n = cfg["N"]; in_dim = cfg["IN"]; hc1 = cfg["HC1"]; h1 = cfg["H1"]
    hid = cfg["HID"]; out_dim = cfg["OUT"]; ncores = cfg["NCORES"]

    x = np.asarray(inputs["x"], np.float32)
    ei = np.asarray(inputs["edge_index"])
    W1 = np.asarray(inputs["W1"], np.float32)
    a_src1 = np.asarray(inputs["a_src1"], np.float32)
    a_dst1 = np.asarray(inputs["a_dst1"], np.float32)
    b1 = np.asarray(inputs["b1"], np.float32)
    W2 = np.asarray(inputs["W2"], np.float32)
    a_src2 = np.asarray(inputs["a_src2"], np.float32)
    a_dst2 = np.asarray(inputs["a_dst2"], np.float32)
    b2 = np.asarray(inputs["b2"], np.float32)

    ntiles = _cdiv(n, P)
    npad = ntiles * P
    xT = np.zeros((in_dim, npad), bf16)
    xT[:, :n] = x.T

    amat = np.zeros((hc1, 2 * h1), np.float32)
    for h in range(h1):
        amat[h * hid:(h + 1) * hid, h] = a_src1[h]
        amat[h * hid:(h + 1) * hid, h1 + h] = a_dst1[h]
    # permute hidden cols from h-major (h*hid+j) to j-major (j*h1+h) so the
    # per-head p broadcast multiplies an inner-contiguous h1-vector on DVE
    jmaj = np.arange(hc1).reshape(hid, h1)
    perm = (jmaj % h1) * hid + jmaj // h1          # new col -> old col
    perm = perm.reshape(-1)
    W1ext = np.concatenate([W1[:, perm], W1 @ amat], axis=1).astype(bf16)
    W2e_full = np.concatenate(
        [W2, (W2 @ a_src2[0])[:, None], (W2 @ a_dst2[0])[:, None]], axis=1)
    W2ext = W2e_full[perm, :].astype(bf16)                         # [256, 66]
    b1 = b1[perm]

    pe = prep_edges(ei, n, ncores)
    for k in ("pairs", "CTlo", "CThi", "CT", "cmaxp"):
        cfg[k] = pe[k]

    common = {
        "W1ext": W1ext, "W2ext": W2ext, "xT": xT,
        "b1r": np.tile(b1[None, :], (P, 1)).astype(np.float32),
        "b2r": np.tile(b2[None, :], (P, 1)).astype(np.float32),
    }
    in_maps = []
    for c in range(ncores):
        m = dict(common)
        m["srclo16"] = np.ascontiguousarray(pe["srclo16"][c])
        m["srchi16"] = np.ascontiguousarray(pe["srchi16"][c])
        m["STh"] = np.ascontiguousarray(pe["ST"][c])
        m["Sh"] = np.ascontiguousarray(pe["S"][c])
        in_maps.append(m)
    return in_maps


DEFAULT_CFG = dict(N=N, IN=IN_DIM, HC1=HC1, H1=H1, HID=HID, OUT=OUT,
                   NCORES=NCORES, NEG=NEG_SLOPE)

TRACE = False
LAST_RESULTS = None


def kernel(**inputs) -> np.ndarray:
    global LAST_RESULTS
    from concourse.bass_utils import run_bass_kernel_spmd

    cfg = dict(DEFAULT_CFG)
    in_maps = make_in_maps(inputs, cfg)
    nc = build_nc(cfg)
    if not nc.is_finalized():
        nc.finalize()
    res = run_bass_kernel_spmd(nc, in_maps, core_ids=list(range(cfg["NCORES"])),
                               trace=TRACE)
    LAST_RESULTS = res
    outs = [res.results[c]["out"] for c in range(cfg["NCORES"])]
    return np.concatenate(outs, axis=0).astype(np.float32)
